# revision 10
# baseline (speedup 1.0000x reference)
"""Multi-head attention (B=4, S=2048, D=1024, H=16, dh=64) on 8 TRN2 NeuronCores.

Sharding: core c = (batch b, head-group g) with b = c // 2, g = c % 2.
Each core computes heads g*8..g*8+7 for batch b.

v16: uniform ctx-defer-by-one-qb pipeline. Each (pass, qb) unit's 16 slots
emit scores+exp for (p, qb) and ctx matmuls for the PREVIOUS unit (reading
its 16 retained e tiles), so attention starts after only x-chunk0 DMA +
Q(qb0) + K(b0) projection (~9us) instead of the full pass-0 projection
prefix (~52us). V projection is single-pass (512-col moving, all 8 head
segs per token block) and fed during pass-0 slots. Normalize uses
reciprocal_approx_fast (5x faster than DVE RECIPROCAL) + the v15 DRAM
bounce for partition broadcast. o-proj: ftiles 0-2 partial into ypart
during pass 3 (evac on DVE/Pool, not ACT), ftile3+add+store pulled into
pass-3/ghost slots; only qb2/qb3 finals remain in the tail.
PSUM: scores 2x2 + ctx 2 + proj ring 2 = 8 banks.
"""
import json
import os
import sys

sys.path.insert(0, "/opt/trn_rl_repo")

import numpy as np
import ml_dtypes

import concourse.bass as bass
import concourse.tile as tile
from concourse import mybir

F32 = mybir.dt.float32
BF16 = mybir.dt.bfloat16
EXP = mybir.ActivationFunctionType.Exp

D = 1024
S = 2048
B = 4
FT = 512
DH = 64
NKT_IN = D // 128     # 8
NQB = S // 512        # 4
NKT = S // 128        # 16
NTT = S // 128        # 16
SCALE = 1.0 / 8.0
N_CORES = 8


def _fix_bir_json(bir_bytes):
    j = json.loads(bir_bytes)
    n = 0
    for fn in j["functions"]:
        for blk in fn["blocks"]:
            out = []
            changed = False
            for inst in blk["instructions"]:
                si = inst.get("sync_info")
                waits = si.get("on_wait") if si else None
                if waits and len(waits) > 1:
                    for w in waits[:-1]:
                        n += 1
                        nop = {
                            "name": f"I-wsplit-{n}",
                            "opcode": "NoOp",
                            "engine": inst["engine"],
                            "ins": [],
                            "outs": [],
                            "sync_info": {"on_wait": [w], "on_update": []},
                        }
                        if "debug" in inst:
                            nop["debug"] = inst["debug"]
                        out.append(nop)
                    si["on_wait"] = [waits[-1]]
                    changed = True
                out.append(inst)
            if changed:
                blk["instructions"] = out
    return json.dumps(j).encode()


def _install_compile_patch():
    import concourse.bass_utils as _bu
    import concourse.bass2jax as _b2j

    if getattr(_bu, "_waitfix_installed", False):
        return
    _orig = _bu.compile_bir_kernel

    def _patched(bir_json, tmpdir, neff_name="file.neff"):
        return _orig(_fix_bir_json(bir_json), tmpdir, neff_name)

    _bu.compile_bir_kernel = _patched
    _b2j.compile_bir_kernel = _patched
    _bu._waitfix_installed = True


def _build():
    nc = bass.Bass("TRN2", target_bir_lowering=False, debug=False,
                   enable_asserts=False, num_devices=N_CORES)

    xT = nc.dram_tensor("xT", [D, S], BF16, kind="ExternalInput")
    wq = nc.dram_tensor("wq", [D, FT], BF16, kind="ExternalInput")
    wk = nc.dram_tensor("wk", [D, FT], BF16, kind="ExternalInput")
    wv = nc.dram_tensor("wv", [D, FT], BF16, kind="ExternalInput")
    wo = nc.dram_tensor("wo", [FT, D], BF16, kind="ExternalInput")
    bq = nc.dram_tensor("bq", [FT], F32, kind="ExternalInput")
    bk = nc.dram_tensor("bk", [FT], F32, kind="ExternalInput")
    bvones = nc.dram_tensor("bvones", [8 * 66], BF16, kind="ExternalInput")
    yT = nc.dram_tensor("yT", [D, S], BF16, kind="ExternalOutput")
    rscr = nc.dram_tensor("rscr", [64, 512], F32, kind="Internal")

    with tile.TileContext(nc) as tc:
        with tc.tile_pool(name="qk_sb", bufs=2) as qk_sb, \
             tc.tile_pool(name="v1_sb", bufs=1) as v1_sb, \
             tc.tile_pool(name="ctxn_sb", bufs=1) as ctxn_sb, \
             tc.tile_pool(name="x_sb", bufs=1) as x_sb, \
             tc.tile_pool(name="w_sb", bufs=2) as w_sb, \
             tc.tile_pool(name="wv_sb", bufs=1) as wv_sb, \
             tc.tile_pool(name="wo_sb", bufs=1) as wo_sb, \
             tc.tile_pool(name="b_sb", bufs=2) as b_sb, \
             tc.tile_pool(name="e_sb", bufs=18) as e_sb, \
             tc.tile_pool(name="r_sb", bufs=2) as r_sb, \
             tc.tile_pool(name="y_sb", bufs=3) as y_sb, \
             tc.tile_pool(name="yp_sb", bufs=1) as yp_sb, \
             tc.tile_pool(name="ps_s", bufs=2, space="PSUM") as ps_s, \
             tc.tile_pool(name="ps_c", bufs=1, space="PSUM") as ps_c, \
             tc.tile_pool(name="ps_p", bufs=2, space="PSUM") as ps_p:

            ctxn = [ctxn_sb.tile([128, S], BF16, tag=f"ctxn{i}", name=f"ctxn{i}")
                    for i in range(4)]

            # ---- weight loads (gpsimd queue; wq/wk first for Q(qb0)/K(b0)) ----
            def load_qk_weights(p):
                wq_t, wk_t = [], []
                for kt in range(NKT_IN):
                    tq = w_sb.tile([128, 128], BF16, tag=f"wq{kt}")
                    nc.gpsimd.dma_start(tq[:], wq.ap()[kt * 128:(kt + 1) * 128,
                                                       p * 128:(p + 1) * 128])
                    wq_t.append(tq)
                    tk = w_sb.tile([128, 128], BF16, tag=f"wk{kt}")
                    nc.gpsimd.dma_start(tk[:], wk.ap()[kt * 128:(kt + 1) * 128,
                                                       p * 128:(p + 1) * 128])
                    wk_t.append(tk)
                bq_t = b_sb.tile([128, 1], F32, tag="bq")
                nc.gpsimd.dma_start(bq_t[:], bq.ap()[p * 128:(p + 1) * 128][:, None])
                bk_t = b_sb.tile([128, 1], F32, tag="bk")
                nc.gpsimd.dma_start(bk_t[:], bk.ap()[p * 128:(p + 1) * 128][:, None])
                return wq_t, wk_t, bq_t, bk_t

            w0 = load_qk_weights(0)

            # ---- x: chunked loads, chunk 0 (tokens 0:512) first ----
            xts = [x_sb.tile([128, S], BF16, tag=f"x{kt}", name=f"x{kt}")
                   for kt in range(NKT_IN)]
            for c in range(4):
                for kt in range(NKT_IN):
                    nc.sync.dma_start(xts[kt][:, c * 512:(c + 1) * 512],
                                      xT.ap()[kt * 128:(kt + 1) * 128,
                                              c * 512:(c + 1) * 512])

            # V weights (single-pass: full [128,512] per kt) + packed bias
            wv_t = []
            for kt in range(NKT_IN):
                t = wv_sb.tile([128, 512], BF16, tag=f"wv{kt}")
                nc.gpsimd.dma_start(t[:], wv.ap()[kt * 128:(kt + 1) * 128, :])
                wv_t.append(t)
            bb = b_sb.tile([128, 528], BF16, tag="bb")
            nc.gpsimd.dma_start(bb[:], bass.AP(
                tensor=bvones, offset=0, ap=[[0, 128], [1, 528]]))

            v1 = [v1_sb.tile([128, 528], BF16, tag=f"v1_{tt}", name=f"v1_{tt}")
                  for tt in range(NTT)]

            # ---- item generators (each item = one PE matmul + tail ops) ----
            def qk_items(wq_t, wk_t, bq_t, bk_t, qt_dst, kt_dst, order):
                for which, qb in order:
                    wt, bt, dst = ((wq_t, bq_t, qt_dst) if which == "q"
                                   else (wk_t, bk_t, kt_dst))
                    pp = ps_p.tile([128, 512], F32, tag="pj", name="pp")

                    def emit(kt, pp=pp, wt=wt, bt=bt, dst=dst, qb=qb):
                        nc.tensor.matmul(pp[:], wt[kt][:],
                                         xts[kt][:, qb * 512:(qb + 1) * 512],
                                         start=(kt == 0), stop=(kt == NKT_IN - 1))
                        if kt == NKT_IN - 1:
                            nc.vector.tensor_scalar_add(
                                dst[:, qb * 512:(qb + 1) * 512], pp[:], bt[:])

                    for kt in range(NKT_IN):
                        yield lambda kt=kt, emit=emit: emit(kt)

            def v_items(tts):
                for tt in tts:
                    pv = ps_p.tile([128, 512], F32, tag="pj", name="pv")

                    def emit(kt, pv=pv, tt=tt):
                        nc.tensor.matmul(pv[:], xts[kt][:, tt * 128:(tt + 1) * 128],
                                         wv_t[kt][:],
                                         start=(kt == 0), stop=(kt == NKT_IN - 1))
                        if kt == NKT_IN - 1:
                            v1v = v1[tt][:].rearrange("p (s c) -> p s c", c=66)
                            nc.vector.tensor_add(
                                v1v[:, :, 0:64],
                                pv[:].rearrange("p (s c) -> p s c", c=64),
                                bb[:].rearrange("p (s c) -> p s c", c=66)[:, :, 0:64])
                            nc.gpsimd.memset(v1v[:, :, 64:65], 1.0)

                    for kt in range(NKT_IN):
                        yield lambda kt=kt, emit=emit: emit(kt)

            wo_t = []
            ypart = []

            def oproj_partial_items():
                """ftiles 0-2 accumulated into ypart. 4 qb x 8 ot x 3 = 96."""
                ecnt = [0]
                for qb in range(NQB):
                    for ot in range(8):
                        yp = ps_p.tile([128, 512], F32, tag="pj", name="yp")

                        def emit(ftile, yp=yp, ot=ot, qb=qb):
                            nc.tensor.matmul(
                                yp[:],
                                wo_t[ftile][:, ot * 128:(ot + 1) * 128],
                                ctxn[ftile][:, qb * 512:(qb + 1) * 512],
                                start=(ftile == 0), stop=(ftile == 2))
                            if ftile == 2:
                                nc.vector.tensor_copy(
                                    ypart[ot][:, qb * 512:(qb + 1) * 512], yp[:])

                        for ftile in range(3):
                            yield lambda ftile=ftile, emit=emit: emit(ftile)

            def f3_items(qb):
                """ftile3 + add partial + store, one output qb: 8 items."""
                for ot in range(8):
                    def emit(ot=ot, qb=qb):
                        yp = ps_p.tile([128, 512], F32, tag="pj", name="yp3")
                        nc.tensor.matmul(yp[:],
                                         wo_t[3][:, ot * 128:(ot + 1) * 128],
                                         ctxn[3][:, qb * 512:(qb + 1) * 512],
                                         start=True, stop=True)
                        ys = y_sb.tile([128, 512], BF16, tag="ys")
                        nc.vector.tensor_add(
                            ys[:], yp[:],
                            ypart[ot][:, qb * 512:(qb + 1) * 512])
                        nc.scalar.dma_start(
                            yT.ap()[ot * 128:(ot + 1) * 128,
                                    qb * 512:(qb + 1) * 512], ys[:])
                    yield emit

            # ---- prefix: Q(qb0), K(b0), V(tt0..3) ----
            qt_t = qk_sb.tile([128, S], BF16, tag="qt")
            kt_t = qk_sb.tile([128, S], BF16, tag="kt")
            for it in qk_items(*w0, qt_t, kt_t, [("q", 0), ("k", 0)]):
                it()

            # ---- global feed queue ----
            from collections import deque
            feeds = deque()

            def extend(gen):
                feeds.extend(gen)

            # pass-0 feeds (deadline order), then pass-1 QK prefetch
            extend(qk_items(*w0, qt_t, kt_t, [("k", 1), ("k", 2)]))
            extend(v_items(range(0, 2)))
            extend(qk_items(*w0, qt_t, kt_t, [("k", 3)]))
            extend(v_items(range(2, 4)))
            extend(qk_items(*w0, qt_t, kt_t, [("q", 1)]))
            extend(v_items(range(4, 16)))
            extend(qk_items(*w0, qt_t, kt_t, [("q", 2), ("q", 3)]))

            pending_norm = []
            rbs = []

            def push_norm(cA, cB, pp, qq):
                for h, cx in ((0, cA), (1, cB)):
                    slot = (2 * pp + h) * 4 + qq

                    def n_recip(cx=cx, slot=slot):
                        nc.sync.dma_start(
                            rscr.ap()[slot:slot + 1, :], cx[64:65, :])
                        d8 = r_sb.tile([64, 8], F32, tag="d8")
                        nc.gpsimd.dma_start(d8[:], bass.AP(
                            tensor=rscr, offset=slot * 512,
                            ap=[[8, 64], [1, 8]]))
                        r8 = r_sb.tile([64, 8], F32, tag="r8")
                        nc.vector.reciprocal(r8[:], d8[:])
                        nc.sync.dma_start(bass.AP(
                            tensor=rscr, offset=(32 + slot) * 512,
                            ap=[[8, 64], [1, 8]]), r8[:])
                        return slot

                    def n_mul(h=h, cx=cx, pp=pp, qq=qq):
                        slot = rbs.pop(0)
                        rb = r_sb.tile([64, 512], F32, tag="rb")
                        nc.gpsimd.dma_start(rb[:], bass.AP(
                            tensor=rscr, offset=(32 + slot) * 512,
                            ap=[[0, 64], [1, 512]]))
                        if h == 0:
                            nc.gpsimd.tensor_mul(
                                ctxn[pp][0:64, qq * 512:(qq + 1) * 512],
                                cx[0:64, :], rb[:])
                        else:
                            cn = r_sb.tile([64, 512], BF16, tag="cn")
                            nc.gpsimd.tensor_mul(cn[:], cx[0:64, :], rb[:])
                            nc.sync.dma_start(
                                ctxn[pp][64:128, qq * 512:(qq + 1) * 512],
                                cn[:])

                    def n_first(n_recip=n_recip):
                        rbs.append(n_recip())

                    pending_norm.append(n_first)
                    pending_norm.append(n_mul)

            def quota(p, qb, kt):
                if p == 0:
                    return (5, 6, 2, 2)[qb]
                if p == 3:
                    return 1 + (kt % 2)
                return 1

            # ---- main loop: 16 units + ghost ----
            units = [(p, qb) for p in range(4) for qb in range(NQB)]
            prev = None            # (p, qb, e_list)
            e_cur = None
            qt_n = kt_n = None

            for ui, (p, qb) in enumerate(units):
                if qb == 0 and p > 0:
                    qt_t, kt_t = qt_n, kt_n
                if qb == 0 and p < 3:
                    # prefetch next pass Q/K: weights + destination tiles
                    wn = load_qk_weights(p + 1)
                    qt_n = qk_sb.tile([128, S], BF16, tag="qt")
                    kt_n = qk_sb.tile([128, S], BF16, tag="kt")
                    extend(qk_items(*wn, qt_n, kt_n,
                                    [("q", 0), ("k", 0), ("k", 1), ("k", 2),
                                     ("k", 3), ("q", 1), ("q", 2), ("q", 3)]))
                if p == 2 and qb == 0:
                    for ftile in range(4):
                        t = wo_sb.tile([128, 1024], BF16, tag=f"wo{ftile}")
                        nc.gpsimd.dma_start(
                            t[:], wo.ap()[ftile * 128:(ftile + 1) * 128, :])
                        wo_t.append(t)
                    ypart = [yp_sb.tile([128, S], BF16, tag=f"ypart{ot}",
                                        name=f"ypart{ot}") for ot in range(8)]
                if p == 3 and qb == 0:
                    extend(oproj_partial_items())
                if p == 3 and qb == 3:
                    extend(f3_items(0))

                e_new = []
                if prev is not None:
                    pp_, pq_ = prev[0], prev[1]
                    ctxA = ps_c.tile([65, 512], F32, tag="ctxA")
                    ctxB = ps_c.tile([65, 512], F32, tag="ctxB")
                for kt in range(NKT):
                    if kt in (1, 7, 8, 15) and pending_norm:
                        pending_norm.pop(0)()
                    # scores + exp for (p, qb, kt)
                    sp = ps_s.tile([128, 1024], F32, tag="sp")
                    nc.tensor.matmul(
                        sp[:, 0:512],
                        kt_t[0:64, kt * 128:(kt + 1) * 128],
                        qt_t[0:64, qb * 512:(qb + 1) * 512],
                        start=True, stop=True)
                    nc.tensor.matmul(
                        sp[:, 512:1024],
                        kt_t[64:128, kt * 128:(kt + 1) * 128],
                        qt_t[64:128, qb * 512:(qb + 1) * 512],
                        start=True, stop=True)
                    e_t = e_sb.tile([128, 1024], BF16, tag="e")
                    nc.scalar.activation(e_t[:], sp[:], EXP, scale=SCALE)
                    e_new.append(e_t)
                    # feeds before ctx in pass 0 (V deadline pressure)
                    nfeed = quota(p, qb, kt)
                    if p == 0:
                        for _ in range(nfeed):
                            if feeds:
                                feeds.popleft()()
                    # ctx for previous unit
                    if prev is not None:
                        ep = prev[2][kt]
                        v1v = v1[kt][:].rearrange("p (s c) -> p s c", c=66)
                        nc.tensor.matmul(ctxA[:], v1v[:, 2 * pp_, 0:65],
                                         ep[:, 0:512],
                                         start=(kt == 0), stop=(kt == NKT - 1))
                        nc.tensor.matmul(ctxB[:], v1v[:, 2 * pp_ + 1, 0:65],
                                         ep[:, 512:1024],
                                         start=(kt == 0), stop=(kt == NKT - 1))
                    if p != 0:
                        for _ in range(nfeed):
                            if feeds:
                                feeds.popleft()()
                # end of unit: evacuate prev ctx, queue normalize
                if prev is not None:
                    cA = r_sb.tile([65, 512], F32, tag="cA")
                    cB = r_sb.tile([65, 512], F32, tag="cB")
                    nc.vector.tensor_copy(cA[:], ctxA[:])
                    nc.vector.tensor_copy(cB[:], ctxB[:])
                    push_norm(cA, cB, prev[0], prev[1])
                prev = (p, qb, e_new)

            # ---- ghost phase: ctx for (3,3) compactly + norms early + f3 ----
            extend(f3_items(1))
            ctxA = ps_c.tile([65, 512], F32, tag="ctxA")
            ctxB = ps_c.tile([65, 512], F32, tag="ctxB")
            for kt in range(NKT):
                if kt in (1, 3, 5, 7) and pending_norm:
                    pending_norm.pop(0)()
                if kt == 8:
                    extend(f3_items(2))
                ep = prev[2][kt]
                v1v = v1[kt][:].rearrange("p (s c) -> p s c", c=66)
                nc.tensor.matmul(ctxA[:], v1v[:, 6, 0:65], ep[:, 0:512],
                                 start=(kt == 0), stop=(kt == NKT - 1))
                nc.tensor.matmul(ctxB[:], v1v[:, 7, 0:65], ep[:, 512:1024],
                                 start=(kt == 0), stop=(kt == NKT - 1))
                for _ in range(2):
                    if feeds:
                        feeds.popleft()()

            # ---- tail: evac + normalize (3,3), finals for qb2/qb3 ----
            while feeds:
                feeds.popleft()()
            cA = r_sb.tile([65, 512], F32, tag="cA")
            cB = r_sb.tile([65, 512], F32, tag="cB")
            nc.vector.tensor_copy(cA[:], ctxA[:])
            nc.vector.tensor_copy(cB[:], ctxB[:])
            # recips + broadcast DMAs first so latency overlaps f3(qb2)
            rb_tail = []
            for h, cx in ((0, cA), (1, cB)):
                slot = (6 + h) * 4 + 3
                nc.sync.dma_start(rscr.ap()[slot:slot + 1, :], cx[64:65, :])
                d8 = r_sb.tile([64, 8], F32, tag="d8")
                nc.gpsimd.dma_start(d8[:], bass.AP(
                    tensor=rscr, offset=slot * 512, ap=[[8, 64], [1, 8]]))
                r8 = r_sb.tile([64, 8], F32, tag="r8")
                nc.vector.reciprocal(r8[:], d8[:])
                nc.sync.dma_start(bass.AP(
                    tensor=rscr, offset=(32 + slot) * 512,
                    ap=[[8, 64], [1, 8]]), r8[:])
                rb = r_sb.tile([64, 512], F32, tag="rb")
                nc.gpsimd.dma_start(rb[:], bass.AP(
                    tensor=rscr, offset=(32 + slot) * 512,
                    ap=[[0, 64], [1, 512]]))
                rb_tail.append(rb)
            while feeds:
                feeds.popleft()()
            nc.gpsimd.tensor_mul(ctxn[3][0:64, 3 * 512:4 * 512],
                                 cA[0:64, :], rb_tail[0][:])
            cn = r_sb.tile([64, 512], BF16, tag="cn")
            nc.gpsimd.tensor_mul(cn[:], cB[0:64, :], rb_tail[1][:])
            nc.sync.dma_start(ctxn[3][64:128, 3 * 512:4 * 512], cn[:])
            for it in f3_items(3):
                it()
    return nc


_nc_cache = None


def _get_nc():
    global _nc_cache
    if _nc_cache is None:
        _install_compile_patch()
        _nc_cache = _build()
    return _nc_cache


def _execute(inputs, trace=False, tmpdir=None):
    from concourse.bass_utils import run_bass_kernel_spmd

    bf16 = ml_dtypes.bfloat16
    x = np.asarray(inputs["x"], dtype=np.float32)
    Wq = np.asarray(inputs["Wq"], dtype=np.float32).astype(bf16)
    Wk = np.asarray(inputs["Wk"], dtype=np.float32).astype(bf16)
    Wv = np.asarray(inputs["Wv"], dtype=np.float32).astype(bf16)
    Wo = np.asarray(inputs["Wo"], dtype=np.float32).astype(bf16)
    bq = np.asarray(inputs["bq"], dtype=np.float32)
    bk = np.asarray(inputs["bk"], dtype=np.float32)
    bv = np.asarray(inputs["bv"], dtype=np.float32)
    bo = np.asarray(inputs["bo"], dtype=np.float32)

    nc = _get_nc()
    in_maps = []
    for c in range(N_CORES):
        b, g = c // 2, c % 2
        sl = slice(g * FT, (g + 1) * FT)
        bv_g = bv[sl].reshape(8, 64)
        bvones = np.concatenate(
            [bv_g, np.ones((8, 1), np.float32), np.zeros((8, 1), np.float32)],
            axis=1).reshape(-1)
        in_maps.append({
            "xT": np.ascontiguousarray(x[b].T).astype(bf16),
            "wq": np.ascontiguousarray(Wq[:, sl]),
            "wk": np.ascontiguousarray(Wk[:, sl]),
            "wv": np.ascontiguousarray(Wv[:, sl]),
            "wo": np.ascontiguousarray(Wo[sl, :]),
            "bq": np.ascontiguousarray(bq[sl]),
            "bk": np.ascontiguousarray(bk[sl]),
            "bvones": bvones.astype(bf16),
        })

    kwargs = {}
    if trace:
        kwargs = dict(trace=True, tmpdir=tmpdir)
    res = run_bass_kernel_spmd(nc, in_maps, core_ids=list(range(N_CORES)), **kwargs)

    out = np.empty((B, S, D), dtype=np.float32)
    for b in range(B):
        yT0 = res.results[2 * b]["yT"].astype(np.float32)
        yT1 = res.results[2 * b + 1]["yT"].astype(np.float32)
        out[b] = (yT0 + yT1).T + bo
    return out, res


def kernel(**inputs) -> np.ndarray:
    out, _ = _execute(inputs, trace=False)
    return out


# revision 12
# speedup vs baseline: 1.0148x; 1.0148x over previous
"""Multi-head attention (B=4, S=2048, D=1024, H=16, dh=64) on 8 TRN2 NeuronCores.

Sharding: core c = (batch b, head-group g) with b = c // 2, g = c % 2.
Each core computes heads g*8..g*8+7 for batch b.

v16: uniform ctx-defer-by-one-qb pipeline. Each (pass, qb) unit's 16 slots
emit scores+exp for (p, qb) and ctx matmuls for the PREVIOUS unit (reading
its 16 retained e tiles), so attention starts after only x-chunk0 DMA +
Q(qb0) + K(b0) projection (~9us) instead of the full pass-0 projection
prefix (~52us). V projection is single-pass (512-col moving, all 8 head
segs per token block) and fed during pass-0 slots. Normalize uses
reciprocal_approx_fast (5x faster than DVE RECIPROCAL) + the v15 DRAM
bounce for partition broadcast. o-proj: ftiles 0-2 partial into ypart
during pass 3 (evac on DVE/Pool, not ACT), ftile3+add+store pulled into
pass-3/ghost slots; only qb2/qb3 finals remain in the tail.
PSUM: scores 2x2 + ctx 2 + proj ring 2 = 8 banks.
"""
import json
import os
import sys

sys.path.insert(0, "/opt/trn_rl_repo")

import numpy as np
import ml_dtypes

import concourse.bass as bass
import concourse.tile as tile
from concourse import mybir

F32 = mybir.dt.float32
BF16 = mybir.dt.bfloat16
EXP = mybir.ActivationFunctionType.Exp

D = 1024
S = 2048
B = 4
FT = 512
DH = 64
NKT_IN = D // 128     # 8
NQB = S // 512        # 4
NKT = S // 128        # 16
NTT = S // 128        # 16
SCALE = 1.0 / 8.0
N_CORES = 8


def _fix_bir_json(bir_bytes):
    j = json.loads(bir_bytes)
    n = 0
    for fn in j["functions"]:
        for blk in fn["blocks"]:
            out = []
            changed = False
            for inst in blk["instructions"]:
                si = inst.get("sync_info")
                waits = si.get("on_wait") if si else None
                if waits and len(waits) > 1:
                    for w in waits[:-1]:
                        n += 1
                        nop = {
                            "name": f"I-wsplit-{n}",
                            "opcode": "NoOp",
                            "engine": inst["engine"],
                            "ins": [],
                            "outs": [],
                            "sync_info": {"on_wait": [w], "on_update": []},
                        }
                        if "debug" in inst:
                            nop["debug"] = inst["debug"]
                        out.append(nop)
                    si["on_wait"] = [waits[-1]]
                    changed = True
                out.append(inst)
            if changed:
                blk["instructions"] = out
    return json.dumps(j).encode()


def _install_compile_patch():
    import concourse.bass_utils as _bu
    import concourse.bass2jax as _b2j

    if getattr(_bu, "_waitfix_installed", False):
        return
    _orig = _bu.compile_bir_kernel

    def _patched(bir_json, tmpdir, neff_name="file.neff"):
        return _orig(_fix_bir_json(bir_json), tmpdir, neff_name)

    _bu.compile_bir_kernel = _patched
    _b2j.compile_bir_kernel = _patched
    _bu._waitfix_installed = True


def _build():
    nc = bass.Bass("TRN2", target_bir_lowering=False, debug=False,
                   enable_asserts=False, num_devices=N_CORES)

    xT = nc.dram_tensor("xT", [D, S], BF16, kind="ExternalInput")
    wq = nc.dram_tensor("wq", [D, FT], BF16, kind="ExternalInput")
    wk = nc.dram_tensor("wk", [D, FT], BF16, kind="ExternalInput")
    wv = nc.dram_tensor("wv", [D, FT], BF16, kind="ExternalInput")
    wo = nc.dram_tensor("wo", [FT, D], BF16, kind="ExternalInput")
    bq = nc.dram_tensor("bq", [FT], F32, kind="ExternalInput")
    bk = nc.dram_tensor("bk", [FT], F32, kind="ExternalInput")
    bvones = nc.dram_tensor("bvones", [8 * 66], BF16, kind="ExternalInput")
    yT = nc.dram_tensor("yT", [D, S], BF16, kind="ExternalOutput")
    rscr = nc.dram_tensor("rscr", [64, 512], F32, kind="Internal")

    with tile.TileContext(nc) as tc:
        with tc.tile_pool(name="qk_sb", bufs=2) as qk_sb, \
             tc.tile_pool(name="v1_sb", bufs=1) as v1_sb, \
             tc.tile_pool(name="ctxn_sb", bufs=1) as ctxn_sb, \
             tc.tile_pool(name="x_sb", bufs=1) as x_sb, \
             tc.tile_pool(name="w_sb", bufs=2) as w_sb, \
             tc.tile_pool(name="wv_sb", bufs=1) as wv_sb, \
             tc.tile_pool(name="wo_sb", bufs=1) as wo_sb, \
             tc.tile_pool(name="b_sb", bufs=2) as b_sb, \
             tc.tile_pool(name="e_sb", bufs=18) as e_sb, \
             tc.tile_pool(name="r_sb", bufs=2) as r_sb, \
             tc.tile_pool(name="y_sb", bufs=4) as y_sb, \
             tc.tile_pool(name="yp_sb", bufs=1) as yp_sb, \
             tc.tile_pool(name="ps_s", bufs=2, space="PSUM") as ps_s, \
             tc.tile_pool(name="ps_c", bufs=1, space="PSUM") as ps_c, \
             tc.tile_pool(name="ps_p", bufs=2, space="PSUM") as ps_p:

            ctxn = [ctxn_sb.tile([128, S], BF16, tag=f"ctxn{i}", name=f"ctxn{i}")
                    for i in range(4)]

            # ---- weight loads (gpsimd queue; wq/wk first for Q(qb0)/K(b0)) ----
            def load_qk_weights(p):
                wq_t, wk_t = [], []
                for kt in range(NKT_IN):
                    tq = w_sb.tile([128, 128], BF16, tag=f"wq{kt}")
                    nc.gpsimd.dma_start(tq[:], wq.ap()[kt * 128:(kt + 1) * 128,
                                                       p * 128:(p + 1) * 128])
                    wq_t.append(tq)
                    tk = w_sb.tile([128, 128], BF16, tag=f"wk{kt}")
                    nc.gpsimd.dma_start(tk[:], wk.ap()[kt * 128:(kt + 1) * 128,
                                                       p * 128:(p + 1) * 128])
                    wk_t.append(tk)
                bq_t = b_sb.tile([128, 1], F32, tag="bq")
                nc.gpsimd.dma_start(bq_t[:], bq.ap()[p * 128:(p + 1) * 128][:, None])
                bk_t = b_sb.tile([128, 1], F32, tag="bk")
                nc.gpsimd.dma_start(bk_t[:], bk.ap()[p * 128:(p + 1) * 128][:, None])
                return wq_t, wk_t, bq_t, bk_t

            w0 = load_qk_weights(0)

            # ---- x: chunked loads, chunk 0 (tokens 0:512) first ----
            xts = [x_sb.tile([128, S], BF16, tag=f"x{kt}", name=f"x{kt}")
                   for kt in range(NKT_IN)]
            for c in range(4):
                for kt in range(NKT_IN):
                    nc.sync.dma_start(xts[kt][:, c * 512:(c + 1) * 512],
                                      xT.ap()[kt * 128:(kt + 1) * 128,
                                              c * 512:(c + 1) * 512])

            # V weights (single-pass: full [128,512] per kt) + packed bias
            wv_t = []
            for kt in range(NKT_IN):
                t = wv_sb.tile([128, 512], BF16, tag=f"wv{kt}")
                nc.gpsimd.dma_start(t[:], wv.ap()[kt * 128:(kt + 1) * 128, :])
                wv_t.append(t)
            bb = b_sb.tile([128, 528], BF16, tag="bb")
            nc.gpsimd.dma_start(bb[:], bass.AP(
                tensor=bvones, offset=0, ap=[[0, 128], [1, 528]]))

            v1 = [v1_sb.tile([128, 528], BF16, tag=f"v1_{tt}", name=f"v1_{tt}")
                  for tt in range(NTT)]

            # ---- item generators (each item = one PE matmul + tail ops) ----
            done = {}           # emission-readiness: ("q"|"k", p, qb), ("v", tt)

            def need(key):
                while key not in done:
                    feeds.popleft()()   # raises IndexError if impossible

            def qk_items(pi, wq_t, wk_t, bq_t, bk_t, qt_dst, kt_dst, order):
                for which, qb in order:
                    wt, bt, dst = ((wq_t, bq_t, qt_dst) if which == "q"
                                   else (wk_t, bk_t, kt_dst))
                    pp = ps_p.tile([128, 512], F32, tag="pj", name="pp")

                    def emit(kt, pp=pp, wt=wt, bt=bt, dst=dst, qb=qb,
                             which=which, pi=pi):
                        nc.tensor.matmul(pp[:], wt[kt][:],
                                         xts[kt][:, qb * 512:(qb + 1) * 512],
                                         start=(kt == 0), stop=(kt == NKT_IN - 1))
                        if kt == NKT_IN - 1:
                            nc.vector.tensor_scalar_add(
                                dst[:, qb * 512:(qb + 1) * 512], pp[:], bt[:])
                            done[(which, pi, qb)] = True

                    for kt in range(NKT_IN):
                        yield lambda kt=kt, emit=emit: emit(kt)

            def v_items(tts):
                for tt in tts:
                    pv = ps_p.tile([128, 512], F32, tag="pj", name="pv")

                    def emit(kt, pv=pv, tt=tt):
                        nc.tensor.matmul(pv[:], xts[kt][:, tt * 128:(tt + 1) * 128],
                                         wv_t[kt][:],
                                         start=(kt == 0), stop=(kt == NKT_IN - 1))
                        if kt == NKT_IN - 1:
                            v1v = v1[tt][:].rearrange("p (s c) -> p s c", c=66)
                            nc.vector.tensor_add(
                                v1v[:, :, 0:64],
                                pv[:].rearrange("p (s c) -> p s c", c=64),
                                bb[:].rearrange("p (s c) -> p s c", c=66)[:, :, 0:64])
                            nc.gpsimd.memset(v1v[:, :, 64:65], 1.0)
                            done[("v", tt)] = True

                    for kt in range(NKT_IN):
                        yield lambda kt=kt, emit=emit: emit(kt)

            wo_t = []
            ypart = []

            def oproj_partial_items():
                """ftiles 0-2 accumulated into ypart. 4 qb x 8 ot x 3 = 96."""
                ecnt = [0]
                for qb in range(NQB):
                    for ot in range(8):
                        yp = ps_p.tile([128, 512], F32, tag="pj", name="yp")

                        def emit(ftile, yp=yp, ot=ot, qb=qb):
                            nc.tensor.matmul(
                                yp[:],
                                wo_t[ftile][:, ot * 128:(ot + 1) * 128],
                                ctxn[ftile][:, qb * 512:(qb + 1) * 512],
                                start=(ftile == 0), stop=(ftile == 2))
                            if ftile == 2:
                                nc.vector.tensor_copy(
                                    ypart[ot][:, qb * 512:(qb + 1) * 512], yp[:])

                        for ftile in range(3):
                            yield lambda ftile=ftile, emit=emit: emit(ftile)

            def f3_items(qb):
                """ftile3 + add partial + store, one output qb: 8 items."""
                for ot in range(8):
                    def emit(ot=ot, qb=qb):
                        if ot % 2 == 0:
                            yp = ps_p.tile([128, 512], F32, tag="pj",
                                           name="yp3")[:]
                        else:
                            yp = ps_s.tile([128, 1024], F32, tag="sp",
                                           name="yp3s")[:, 0:512]
                        nc.tensor.matmul(yp,
                                         wo_t[3][:, ot * 128:(ot + 1) * 128],
                                         ctxn[3][:, qb * 512:(qb + 1) * 512],
                                         start=True, stop=True)
                        ys = y_sb.tile([128, 512], BF16, tag="ys")
                        nc.vector.tensor_add(
                            ys[:], yp,
                            ypart[ot][:, qb * 512:(qb + 1) * 512])
                        nc.scalar.dma_start(
                            yT.ap()[ot * 128:(ot + 1) * 128,
                                    qb * 512:(qb + 1) * 512], ys[:])
                    yield emit

            # ---- prefix: Q(qb0), K(b0), V(tt0..3) ----
            qt_t = qk_sb.tile([128, S], BF16, tag="qt")
            kt_t = qk_sb.tile([128, S], BF16, tag="kt")
            for it in qk_items(0, *w0, qt_t, kt_t, [("q", 0), ("k", 0)]):
                it()

            # ---- global feed queue ----
            from collections import deque
            feeds = deque()

            def extend(gen):
                feeds.extend(gen)

            # pass-0 feeds (deadline order), then pass-1 QK prefetch
            extend(qk_items(0, *w0, qt_t, kt_t, [("k", 1)]))
            extend(v_items(range(0, 2)))
            extend(qk_items(0, *w0, qt_t, kt_t, [("k", 2)]))
            extend(v_items(range(2, 4)))
            extend(qk_items(0, *w0, qt_t, kt_t, [("k", 3), ("q", 1)]))
            extend(v_items(range(4, 14)))
            extend(qk_items(0, *w0, qt_t, kt_t, [("q", 2)]))
            extend(v_items(range(14, 16)))
            extend(qk_items(0, *w0, qt_t, kt_t, [("q", 3)]))

            pending_norm = []
            rbs = []

            def push_norm(cA, cB, pp, qq):
                for h, cx in ((0, cA), (1, cB)):
                    slot = (2 * pp + h) * 4 + qq

                    def n_recip(cx=cx, slot=slot):
                        nc.sync.dma_start(
                            rscr.ap()[slot:slot + 1, :], cx[64:65, :])
                        d8 = r_sb.tile([64, 8], F32, tag="d8")
                        nc.gpsimd.dma_start(d8[:], bass.AP(
                            tensor=rscr, offset=slot * 512,
                            ap=[[8, 64], [1, 8]]))
                        r8 = r_sb.tile([64, 8], F32, tag="r8")
                        nc.vector.reciprocal(r8[:], d8[:])
                        nc.sync.dma_start(bass.AP(
                            tensor=rscr, offset=(32 + slot) * 512,
                            ap=[[8, 64], [1, 8]]), r8[:])
                        return slot

                    def n_mul(h=h, cx=cx, pp=pp, qq=qq):
                        slot = rbs.pop(0)
                        rb = r_sb.tile([64, 512], F32, tag="rb")
                        nc.gpsimd.dma_start(rb[:], bass.AP(
                            tensor=rscr, offset=(32 + slot) * 512,
                            ap=[[0, 64], [1, 512]]))
                        if h == 0:
                            nc.gpsimd.tensor_mul(
                                ctxn[pp][0:64, qq * 512:(qq + 1) * 512],
                                cx[0:64, :], rb[:])
                        else:
                            cn = r_sb.tile([64, 512], BF16, tag="cn")
                            nc.gpsimd.tensor_mul(cn[:], cx[0:64, :], rb[:])
                            nc.sync.dma_start(
                                ctxn[pp][64:128, qq * 512:(qq + 1) * 512],
                                cn[:])

                    def n_first(n_recip=n_recip):
                        rbs.append(n_recip())

                    pending_norm.append(n_first)
                    pending_norm.append(n_mul)

            def quota(p, qb, kt):
                if p == 0:
                    return (5, 5, 2, 2)[qb]
                if p == 3:
                    return 1 + (kt % 2)
                return 1

            # ---- main loop: 16 units + ghost ----
            units = [(p, qb) for p in range(4) for qb in range(NQB)]
            prev = None            # (p, qb, e_list)
            e_cur = None
            qt_n = kt_n = None

            for ui, (p, qb) in enumerate(units):
                if qb == 0 and p > 0:
                    qt_t, kt_t = qt_n, kt_n
                if qb == 0 and p < 3:
                    # prefetch next pass Q/K: weights + destination tiles
                    wn = load_qk_weights(p + 1)
                    qt_n = qk_sb.tile([128, S], BF16, tag="qt")
                    kt_n = qk_sb.tile([128, S], BF16, tag="kt")
                    extend(qk_items(p + 1, *wn, qt_n, kt_n,
                                    [("q", 0), ("k", 0), ("k", 1), ("k", 2),
                                     ("k", 3), ("q", 1), ("q", 2), ("q", 3)]))
                if p == 2 and qb == 0:
                    for ftile in range(4):
                        t = wo_sb.tile([128, 1024], BF16, tag=f"wo{ftile}")
                        nc.gpsimd.dma_start(
                            t[:], wo.ap()[ftile * 128:(ftile + 1) * 128, :])
                        wo_t.append(t)
                    ypart = [yp_sb.tile([128, S], BF16, tag=f"ypart{ot}",
                                        name=f"ypart{ot}") for ot in range(8)]
                if p == 3 and qb == 0:
                    extend(oproj_partial_items())
                if p == 3 and qb == 3:
                    extend(f3_items(0))

                e_new = []
                if prev is not None:
                    pp_, pq_ = prev[0], prev[1]
                    ctxA = ps_c.tile([65, 512], F32, tag="ctxA")
                    ctxB = ps_c.tile([65, 512], F32, tag="ctxB")
                for kt in range(NKT):
                    if kt in (1, 7, 8, 15) and pending_norm:
                        pending_norm.pop(0)()
                    # scores + exp for (p, qb, kt)
                    need(("q", p, qb))
                    need(("k", p, kt // 4))
                    sp = ps_s.tile([128, 1024], F32, tag="sp")
                    nc.tensor.matmul(
                        sp[:, 0:512],
                        kt_t[0:64, kt * 128:(kt + 1) * 128],
                        qt_t[0:64, qb * 512:(qb + 1) * 512],
                        start=True, stop=True)
                    nc.tensor.matmul(
                        sp[:, 512:1024],
                        kt_t[64:128, kt * 128:(kt + 1) * 128],
                        qt_t[64:128, qb * 512:(qb + 1) * 512],
                        start=True, stop=True)
                    e_t = e_sb.tile([128, 1024], BF16, tag="e")
                    nc.scalar.activation(e_t[:], sp[:], EXP, scale=SCALE)
                    e_new.append(e_t)
                    # feeds before ctx in pass 0 (V deadline pressure)
                    nfeed = quota(p, qb, kt)
                    if p == 0:
                        for _ in range(nfeed):
                            if feeds:
                                feeds.popleft()()
                    # ctx for previous unit
                    if prev is not None:
                        need(("v", kt))
                        ep = prev[2][kt]
                        v1v = v1[kt][:].rearrange("p (s c) -> p s c", c=66)
                        nc.tensor.matmul(ctxA[:], v1v[:, 2 * pp_, 0:65],
                                         ep[:, 0:512],
                                         start=(kt == 0), stop=(kt == NKT - 1))
                        nc.tensor.matmul(ctxB[:], v1v[:, 2 * pp_ + 1, 0:65],
                                         ep[:, 512:1024],
                                         start=(kt == 0), stop=(kt == NKT - 1))
                    if p != 0:
                        for _ in range(nfeed):
                            if feeds:
                                feeds.popleft()()
                # end of unit: evacuate prev ctx, queue normalize
                if prev is not None:
                    cA = r_sb.tile([65, 512], F32, tag="cA")
                    cB = r_sb.tile([65, 512], F32, tag="cB")
                    nc.vector.tensor_copy(cA[:], ctxA[:])
                    nc.vector.tensor_copy(cB[:], ctxB[:])
                    push_norm(cA, cB, prev[0], prev[1])
                prev = (p, qb, e_new)

            # ---- ghost phase: ctx for (3,3) compactly + norms early + f3 ----
            extend(f3_items(1))
            ctxA = ps_c.tile([65, 512], F32, tag="ctxA")
            ctxB = ps_c.tile([65, 512], F32, tag="ctxB")
            for kt in range(NKT):
                if kt in (1, 3, 5, 7) and pending_norm:
                    pending_norm.pop(0)()
                if kt == 8:
                    extend(f3_items(2))
                ep = prev[2][kt]
                v1v = v1[kt][:].rearrange("p (s c) -> p s c", c=66)
                nc.tensor.matmul(ctxA[:], v1v[:, 6, 0:65], ep[:, 0:512],
                                 start=(kt == 0), stop=(kt == NKT - 1))
                nc.tensor.matmul(ctxB[:], v1v[:, 7, 0:65], ep[:, 512:1024],
                                 start=(kt == 0), stop=(kt == NKT - 1))
                for _ in range(2):
                    if feeds:
                        feeds.popleft()()

            # ---- tail: evac + normalize (3,3), finals for qb2/qb3 ----
            while feeds:
                feeds.popleft()()
            cA = r_sb.tile([65, 512], F32, tag="cA")
            cB = r_sb.tile([65, 512], F32, tag="cB")
            nc.vector.tensor_copy(cA[:], ctxA[:])
            nc.vector.tensor_copy(cB[:], ctxB[:])
            # recips + broadcast DMAs first so latency overlaps f3(qb2)
            rb_tail = []
            for h, cx in ((0, cA), (1, cB)):
                slot = (6 + h) * 4 + 3
                nc.sync.dma_start(rscr.ap()[slot:slot + 1, :], cx[64:65, :])
                d8 = r_sb.tile([64, 8], F32, tag="d8")
                nc.gpsimd.dma_start(d8[:], bass.AP(
                    tensor=rscr, offset=slot * 512, ap=[[8, 64], [1, 8]]))
                r8 = r_sb.tile([64, 8], F32, tag="r8")
                nc.vector.reciprocal(r8[:], d8[:])
                nc.sync.dma_start(bass.AP(
                    tensor=rscr, offset=(32 + slot) * 512,
                    ap=[[8, 64], [1, 8]]), r8[:])
                rb = r_sb.tile([64, 512], F32, tag="rb")
                nc.gpsimd.dma_start(rb[:], bass.AP(
                    tensor=rscr, offset=(32 + slot) * 512,
                    ap=[[0, 64], [1, 512]]))
                rb_tail.append(rb)
            while feeds:
                feeds.popleft()()
            nc.gpsimd.tensor_mul(ctxn[3][0:64, 3 * 512:4 * 512],
                                 cA[0:64, :], rb_tail[0][:])
            cn = r_sb.tile([64, 512], BF16, tag="cn")
            nc.gpsimd.tensor_mul(cn[:], cB[0:64, :], rb_tail[1][:])
            nc.sync.dma_start(ctxn[3][64:128, 3 * 512:4 * 512], cn[:])
            for it in f3_items(3):
                it()
    return nc


_nc_cache = None


def _get_nc():
    global _nc_cache
    if _nc_cache is None:
        _install_compile_patch()
        _nc_cache = _build()
    return _nc_cache


def _execute(inputs, trace=False, tmpdir=None):
    from concourse.bass_utils import run_bass_kernel_spmd

    bf16 = ml_dtypes.bfloat16
    x = np.asarray(inputs["x"], dtype=np.float32)
    Wq = np.asarray(inputs["Wq"], dtype=np.float32).astype(bf16)
    Wk = np.asarray(inputs["Wk"], dtype=np.float32).astype(bf16)
    Wv = np.asarray(inputs["Wv"], dtype=np.float32).astype(bf16)
    Wo = np.asarray(inputs["Wo"], dtype=np.float32).astype(bf16)
    bq = np.asarray(inputs["bq"], dtype=np.float32)
    bk = np.asarray(inputs["bk"], dtype=np.float32)
    bv = np.asarray(inputs["bv"], dtype=np.float32)
    bo = np.asarray(inputs["bo"], dtype=np.float32)

    nc = _get_nc()
    in_maps = []
    for c in range(N_CORES):
        b, g = c // 2, c % 2
        sl = slice(g * FT, (g + 1) * FT)
        bv_g = bv[sl].reshape(8, 64)
        bvones = np.concatenate(
            [bv_g, np.ones((8, 1), np.float32), np.zeros((8, 1), np.float32)],
            axis=1).reshape(-1)
        in_maps.append({
            "xT": np.ascontiguousarray(x[b].T).astype(bf16),
            "wq": np.ascontiguousarray(Wq[:, sl]),
            "wk": np.ascontiguousarray(Wk[:, sl]),
            "wv": np.ascontiguousarray(Wv[:, sl]),
            "wo": np.ascontiguousarray(Wo[sl, :]),
            "bq": np.ascontiguousarray(bq[sl]),
            "bk": np.ascontiguousarray(bk[sl]),
            "bvones": bvones.astype(bf16),
        })

    kwargs = {}
    if trace:
        kwargs = dict(trace=True, tmpdir=tmpdir)
    res = run_bass_kernel_spmd(nc, in_maps, core_ids=list(range(N_CORES)), **kwargs)

    out = np.empty((B, S, D), dtype=np.float32)
    for b in range(B):
        yT0 = res.results[2 * b]["yT"].astype(np.float32)
        yT1 = res.results[2 * b + 1]["yT"].astype(np.float32)
        out[b] = (yT0 + yT1).T + bo
    return out, res


def kernel(**inputs) -> np.ndarray:
    out, _ = _execute(inputs, trace=False)
    return out
